# revision 1
# baseline (speedup 1.0000x reference)
"""CrossBlock (LightGlue-style dual-softmax cross-attention block) on 8 TRN2 cores.

Data-parallel over batch B=8: one batch element per NeuronCore, weights
replicated. Per-core plan (L=2048 tokens, C=256, H=4 heads, D=64):

  - Activations chained feature-major ("T" = [feature, token]) through the
    PE; weights are the stationary operand, except where token-major output
    is wanted (then the transposed activation tile is stationary).
  - fp32r (full-rate fp32) for projection/FFN matmuls; bf16 for the big
    attention matmuls (sim, attn @ V).
  - Softmax without max-subtraction (logits are ~N(0,1), |sim| < 10 checked
    empirically) -> exp on ScalarE with accum_out giving row-sums for free.
  - Pass A (per head, row tiles): sim = qk0^T-tile @ qk1 -> exp -> P;
    m1 accumulated with ones-augmented v0 (denominator rides as row 64).
  - Pass B (per head, col tiles): simT with a rank-1 augmentation
    (ones x -ln(rowsum), split hi/lo across two K-rows for bf16 accuracy)
    so exp directly yields normalized attn01^T; m0 comes out normalized.
  - m1 normalized via PE-transpose to token-major + gpsimd.normalize_recip
    (denominator rides the transpose as column 64).
  - FFN token-major: LayerNorm stats on DVE (bn_stats), per-token scale via
    per-partition scalar ops, exact-erf GELU on ScalarE, transpose back for
    the W2 matmul, residual + store token-major.
"""

import numpy as np
from contextlib import ExitStack

import concourse.bass as bass
import concourse.tile as tile
from concourse import bacc, mybir
from concourse.bass_utils import run_bass_kernel_spmd
from concourse.masks import make_identity

F32 = mybir.dt.float32
F32R = mybir.dt.float32r
BF16 = mybir.dt.bfloat16
AF = mybir.ActivationFunctionType
ALU = mybir.AluOpType

B, L, C, H = 8, 2048, 256, 4
D = C // H            # 64
C2 = 2 * C            # 512
P = 128
NT = L // P           # 16 token tiles
KC = C // P           # 2 input-feature chunks
KC2 = C2 // P         # 4
SCALE = float(D) ** -0.25
EPS = 1e-5


def r32(ap):
    return ap.bitcast(F32R)


def cross_block(ctx: ExitStack, tc: tile.TileContext, ins, outs):
    nc = tc.nc

    persist = ctx.enter_context(tc.tile_pool(name="persist", bufs=1))
    small = ctx.enter_context(tc.tile_pool(name="small", bufs=2))

    # ---------------- constants / weights ----------------
    ident = persist.tile([P, P], F32)
    make_identity(nc, ident)
    ident_bf = persist.tile([P, P], BF16)
    nc.vector.tensor_copy(ident_bf, ident)

    def load_w(name, k, n, dt=F32):
        t = persist.tile([P, k // P, n], F32, name=f"W_{name}")
        nc.sync.dma_start(out=t, in_=ins[name].rearrange("(k p) n -> p k n", p=P))
        if dt == F32:
            return t
        tb = persist.tile([P, k // P, n], dt, name=f"Wb_{name}")
        nc.vector.tensor_copy(tb, t)
        return tb

    Wqk = load_w("Wqk", C, C, BF16)
    Wv = load_w("Wv", C, C, BF16)
    Wout_bf = load_w("Wout", C, C, BF16)
    W1_bf = load_w("W1", C2, C2, BF16)
    W2_bf = load_w("W2", C2, C, BF16)

    def load_bias_pp(name, n):
        # per-partition layout [P, n/P] for feature-major bias
        t = persist.tile([P, n // P], F32, name=f"bpp_{name}")
        nc.sync.dma_start(out=t, in_=ins[name].rearrange("(k p) -> p k", p=P))
        return t

    bqk_pp = load_bias_pp("bqk", C)
    bqk_s = persist.tile([P, KC], F32)
    nc.scalar.mul(bqk_s, bqk_pp, SCALE)
    bout_pp = load_bias_pp("bout", C)

    def load_bcast(name, n):
        t = persist.tile([P, n], F32, name=f"bc_{name}")
        src = ins[name]
        bc = bass.AP(tensor=src.tensor, offset=src.offset, ap=[[0, P]] + list(src.ap))
        nc.gpsimd.dma_start(out=t, in_=bc)
        return t

    eps_t = persist.tile([P, 1], F32)
    nc.vector.memset(eps_t, EPS)
    bv_bc = load_bcast("bv", C)
    b1_bc = load_bcast("b1", C2)
    g_bc = load_bcast("ln_g", C2)
    lb_bc = load_bcast("ln_b", C2)
    b2_bc = load_bcast("b2", C)

    # whole-kernel activations
    xT = [[persist.tile([P, L], BF16, name=f"xT{s}{kc}") for kc in range(KC)]
          for s in range(2)]
    xtok = [[persist.tile([P, C], F32, name=f"xtok{s}{tt}") for tt in range(NT)]
            for s in range(2)]
    m0T_sb = [persist.tile([P, L], BF16, name=f"m0T{kc}") for kc in range(KC)]
    m1T_sb = [persist.tile([P, L], BF16, name=f"m1T{kc}") for kc in range(KC)]
    outT = [[persist.tile([P, L], BF16, name=f"outT{s}{kc}") for kc in range(KC)]
            for s in range(2)]

    # ================= phase 0/1: x load+transpose, projections =============
    with tc.tile_pool(name="attn_sb", bufs=1) as attn_sb:
      with tc.tile_pool(name="ps01", bufs=2, space="PSUM") as ps01, \
           tc.tile_pool(name="wk01", bufs=3) as wk01:

        for s, name in enumerate(("x0", "x1")):
            xin = ins[name]  # [L, C]
            for tt in range(NT):
                xt = xtok[s][tt]
                nc.gpsimd.dma_start(out=xt, in_=xin[tt * P:(tt + 1) * P, :])
                xtb = wk01.tile([P, C], BF16, tag="xtb", name="xtb")
                nc.vector.tensor_copy(xtb, xt)
                for kc in range(KC):
                    pt = ps01.tile([P, P], BF16, tag="xTp", name="xTp")
                    nc.tensor.transpose(pt, xtb[:, kc * P:(kc + 1) * P], ident_bf)
                    nc.scalar.copy(xT[s][kc][:, tt * P:(tt + 1) * P], pt)

        # qkT aug tiles per stream/head: [66, L] bf16.
        # rows 0:64 = qk_h^T (scaled+biased); rows 64,65: aug rows.
        qkT = [[attn_sb.tile([66, L], BF16, name=f"qkT{s}{h}") for h in range(H)]
               for s in range(2)]
        for s in range(2):
            for mc in range(KC):           # output-feature chunk (2 heads)
                for nt in range(4):        # token span of 512
                    ps = ps01.tile([P, 512], F32, tag="proj", name="proj")
                    for kc in range(KC):
                        nc.tensor.matmul(
                            ps, Wqk[:, kc, mc * P:(mc + 1) * P],
                            xT[s][kc][:, nt * 512:(nt + 1) * 512],
                            start=(kc == 0), stop=(kc == KC - 1))
                    for hh in range(2):
                        h = 2 * mc + hh
                        nc.scalar.activation(
                            qkT[s][h][0:D, nt * 512:(nt + 1) * 512],
                            ps[hh * D:(hh + 1) * D, :], AF.Identity,
                            bias=bqk_s[hh * D:(hh + 1) * D, mc:mc + 1], scale=SCALE)
        for s in range(2):
            for h in range(H):
                nc.vector.memset(qkT[s][h][D:D + 2, :], 1.0)

        # v tiles token-major [128, H, 65] bf16 (col 64 = ones)
        vtok = [[attn_sb.tile([P, H, D + 1], BF16, name=f"v{s}{tt}")
                 for tt in range(NT)] for s in range(2)]
        for s in range(2):
            for tt in range(NT):
                ps = ps01.tile([P, C], F32, tag="proj", name="proj")
                for kc in range(KC):
                    nc.tensor.matmul(
                        ps, xT[s][kc][:, tt * P:(tt + 1) * P],
                        Wv[:, kc, :],
                        start=(kc == 0), stop=(kc == KC - 1))
                nc.vector.scalar_tensor_tensor(
                    out=vtok[s][tt][:, :, 0:D],
                    in0=ps.rearrange("p (h d) -> p h d", h=H), scalar=1.0,
                    in1=bv_bc.rearrange("p (h d) -> p h d", h=H),
                    op0=ALU.mult, op1=ALU.add)
                nc.vector.memset(vtok[s][tt][:, :, D:D + 1], 1.0)

      # ================= phase 2: attention ===============================
      s_all = attn_sb.tile([P, H, NT], F32)     # rowsum of exp(sim)
      m1n_tm = [attn_sb.tile([P, H, D], BF16, name=f"m1n{jt}")
                for jt in range(NT)]

      with tc.tile_pool(name="psSim", bufs=2, space="PSUM") as psSim, \
           tc.tile_pool(name="psAcc", bufs=1, space="PSUM") as psAcc, \
           tc.tile_pool(name="m1u_pool", bufs=2) as m1u_pool, \
           tc.tile_pool(name="wkA", bufs=2) as wkA:
          for h in range(H):
              # ---- pass A ----
              m1ps = psAcc.tile([65, L], F32, tag="macc", name="m1aug")
              for it in range(NT):
                  ptile = wkA.tile([P, L], BF16, tag="P", name="P")
                  sp = small.tile([P, 2], F32, tag="sp", name="sp")
                  for half in range(2):
                      sm = psSim.tile([P, 1024], F32, tag="sim", name="sim")
                      for q in range(2):
                          nc.tensor.matmul(
                              sm[:, q * 512:(q + 1) * 512],
                              qkT[0][h][0:D, it * P:(it + 1) * P],
                              qkT[1][h][0:D,
                                        half * 1024 + q * 512:
                                        half * 1024 + (q + 1) * 512],
                              start=True, stop=True)
                      nc.scalar.activation(
                          ptile[:, half * 1024:(half + 1) * 1024], sm, AF.Exp,
                          accum_out=sp[:, half:half + 1])
                      for q in range(2):
                          sl = slice(half * 1024 + q * 512,
                                     half * 1024 + (q + 1) * 512)
                          nc.tensor.matmul(
                              m1ps[:, sl], vtok[0][it][:, h:h + 1, :],
                              ptile[:, sl],
                              start=(it == 0), stop=(it == NT - 1))
                  nc.vector.tensor_reduce(
                      s_all[:, h, it:it + 1], sp,
                      axis=mybir.AxisListType.X, op=ALU.add)
              m1u = m1u_pool.tile([65, L], F32, tag="m1u", name="m1u")
              nc.vector.tensor_copy(m1u, m1ps)
              # m1 normalize: transpose to token-major, divide by col 64
              for jt in range(NT):
                  tp65 = psSim.tile([P, 65], F32, tag="sim", name="m1tp")
                  nc.tensor.transpose(
                      tp65, m1u[:, jt * P:(jt + 1) * P], ident[0:65, 0:65])
                  blk = wkA.tile([P, 65], F32, tag="m1blk", name="m1blk")
                  nc.vector.tensor_copy(blk, tp65)
                  rcp = small.tile([P, 1], F32, tag="rcp", name="rcp")
                  nc.vector.reciprocal(rcp, blk[:, D:D + 1])
                  nc.vector.tensor_scalar_mul(m1n_tm[jt][:, h, :], blk[:, 0:D], rcp)

              # ---- -ln(s) aug rows (hi/lo) onto the i-side rhs ----
              nls = small.tile([P, NT], F32, tag="nls", name="nls")
              nc.scalar.activation(nls, s_all[:, h, :], AF.Ln)
              nc.vector.tensor_scalar_mul(nls, nls, -1.0)
              nls_hi = small.tile([P, NT], BF16, tag="nlshi", name="nlshi")
              nc.vector.tensor_copy(nls_hi, nls)
              nls_lo = small.tile([P, NT], F32, tag="nlslo", name="nlslo")
              nc.vector.tensor_tensor(nls_lo, nls, nls_hi, ALU.subtract)
              nls_lo_bf = small.tile([P, NT], BF16, tag="nlslobf", name="nlslobf")
              nc.vector.tensor_copy(nls_lo_bf, nls_lo)
              for r, rowt in ((D, nls_hi), (D + 1, nls_lo_bf)):
                  tp = psSim.tile([NT, P], BF16, tag="sim", name="nlsT")
                  nc.tensor.transpose(tp, rowt, ident_bf)
                  tsb = small.tile([NT, P], BF16, tag="nlsT_sb", name="nlsT_sb")
                  nc.vector.tensor_copy(tsb, tp)
                  dst = qkT[0][h][r:r + 1, :]
                  dst = bass.AP(tensor=dst.tensor, offset=dst.offset,
                                ap=[list(dst.ap[0]), [P, NT], [1, P]])
                  nc.gpsimd.dma_start(out=dst, in_=tsb)

              # ---- pass B ----
              m0ps = psAcc.tile([D, L], F32, tag="macc", name="m0acc")
              for jt in range(NT):
                  pt = wkA.tile([P, L], BF16, tag="P", name="P")
                  for half in range(2):
                      sm = psSim.tile([P, 1024], F32, tag="sim", name="sim")
                      for q in range(2):
                          nc.tensor.matmul(
                              sm[:, q * 512:(q + 1) * 512],
                              qkT[1][h][:, jt * P:(jt + 1) * P],
                              qkT[0][h][:,
                                        half * 1024 + q * 512:
                                        half * 1024 + (q + 1) * 512],
                              start=True, stop=True)
                      nc.scalar.activation(
                          pt[:, half * 1024:(half + 1) * 1024], sm, AF.Exp)
                      for q in range(2):
                          sl = slice(half * 1024 + q * 512,
                                     half * 1024 + (q + 1) * 512)
                          nc.tensor.matmul(
                              m0ps[:, sl], vtok[1][jt][:, h:h + 1, 0:D],
                              pt[:, sl],
                              start=(jt == 0), stop=(jt == NT - 1))
              nc.scalar.copy(m0T_sb[h // 2][(h % 2) * D:(h % 2 + 1) * D, :], m0ps)

          # ---- m1 transpose back to feature-major ----
          for kc in range(KC):
              for g4 in range(4):
                  ptb = psSim.tile([P, 512], BF16, tag="sim", name="m1Tp")
                  for q in range(4):
                      jt = g4 * 4 + q
                      srcb = wkA.tile([P, P], BF16, tag="m1bf", name="m1bf")
                      nc.vector.tensor_copy(
                          srcb.rearrange("p (h d) -> p h d", h=2),
                          m1n_tm[jt][:, 2 * kc:2 * kc + 2, :])
                      nc.tensor.transpose(ptb[:, q * P:(q + 1) * P], srcb, ident_bf)
                  nc.vector.tensor_copy(
                      m1T_sb[kc][:, g4 * 512:(g4 + 1) * 512], ptb)

    # ================= phase 3: Wout projection =============================
    with tc.tile_pool(name="psW", bufs=2, space="PSUM") as psW:
        for s, mT in ((0, m0T_sb), (1, m1T_sb)):
            for mc in range(KC):
                for nt in range(4):
                    ps = psW.tile([P, 512], F32, tag="proj", name="proj")
                    for kc in range(KC):
                        nc.tensor.matmul(
                            ps, Wout_bf[:, kc, mc * P:(mc + 1) * P],
                            mT[kc][:, nt * 512:(nt + 1) * 512],
                            start=(kc == 0), stop=(kc == KC - 1))
                    nc.scalar.activation(
                        outT[s][mc][:, nt * 512:(nt + 1) * 512], ps, AF.Identity,
                        bias=bout_pp[:, mc:mc + 1])

    # ================= phase 4: FFN + residual ==============================
    with tc.tile_pool(name="psH", bufs=2, space="PSUM") as psH, \
         tc.tile_pool(name="psG", bufs=1, space="PSUM") as psG, \
         tc.tile_pool(name="psY", bufs=2, space="PSUM") as psY, \
         tc.tile_pool(name="wkF", bufs=3) as wkF, \
         tc.tile_pool(name="g0T_sb", bufs=1) as g0T_sb:
        for s in range(2):
            zchunks = [xT[s][0], xT[s][1], outT[s][0], outT[s][1]]
            g0T = [g0T_sb.tile([P, L], BF16, tag=f"g0T{kc}", name=f"g0T{kc}")
                   for kc in range(KC2)]
            gps = [psG.tile([P, 512], BF16, tag=f"g0p{kc}", name=f"g0p{kc}")
                   for kc in range(KC2)]
            for tt in range(NT):
                hp = psH.tile([P, C2], F32, tag="hps", name="hps")
                for kc in range(KC2):
                    nc.tensor.matmul(
                        hp, zchunks[kc][:, tt * P:(tt + 1) * P], W1_bf[:, kc, :],
                        start=(kc == 0), stop=(kc == KC2 - 1))
                hsb = wkF.tile([P, C2], F32, tag="hsb", name="hsb")
                nc.vector.scalar_tensor_tensor(
                    out=hsb, in0=hp, scalar=1.0, in1=b1_bc,
                    op0=ALU.mult, op1=ALU.add)
                stats = small.tile([P, 6], F32, tag="bnst", name="bnst")
                mv = small.tile([P, 2], F32, tag="bnmv", name="bnmv")
                nc.vector.bn_stats(out=stats, in_=hsb)
                nc.vector.bn_aggr(out=mv, in_=stats)
                rstd = small.tile([P, 1], F32, tag="rstd", name="rstd")
                nc.scalar.activation(rstd, mv[:, 1:2], AF.Sqrt, bias=eps_t)
                nc.vector.reciprocal(rstd, rstd)
                t1 = wkF.tile([P, C2], F32, tag="t1", name="t1")
                nc.vector.scalar_tensor_tensor(
                    out=t1, in0=hsb, scalar=mv[:, 0:1], in1=g_bc,
                    op0=ALU.subtract, op1=ALU.mult)
                t2 = wkF.tile([P, C2], F32, tag="t2", name="t2")
                nc.vector.scalar_tensor_tensor(
                    out=t2, in0=t1, scalar=rstd, in1=lb_bc,
                    op0=ALU.mult, op1=ALU.add)
                g0 = wkF.tile([P, C2], BF16, tag="g0", name="g0")
                nc.scalar.activation(g0, t2, AF.Gelu)
                for kc in range(KC2):
                    nc.tensor.transpose(
                        gps[kc][:, (tt % 4) * P:(tt % 4 + 1) * P],
                        g0[:, kc * P:(kc + 1) * P], ident_bf)
                if tt % 4 == 3:
                    for kc in range(KC2):
                        nc.vector.tensor_copy(
                            g0T[kc][:, (tt - 3) * P:(tt + 1) * P], gps[kc])
                        if tt != NT - 1:
                            gps[kc] = psG.tile([P, 512], BF16,
                                               tag=f"g0p{kc}", name=f"g0p{kc}")
            xout = outs["x0_out"] if s == 0 else outs["x1_out"]
            for tt in range(NT):
                yp = psY.tile([P, C], F32, tag="yps", name="yps")
                for kc in range(KC2):
                    nc.tensor.matmul(
                        yp, g0T[kc][:, tt * P:(tt + 1) * P], W2_bf[:, kc, :],
                        start=(kc == 0), stop=(kc == KC2 - 1))
                xrb = wkF.tile([P, C], F32, tag="xrb", name="xrb")
                nc.gpsimd.tensor_add(xrb, xtok[s][tt], b2_bc)
                yo = wkF.tile([P, C], F32, tag="yout", name="yout")
                nc.vector.scalar_tensor_tensor(
                    out=yo, in0=yp, scalar=1.0, in1=xrb,
                    op0=ALU.mult, op1=ALU.add)
                nc.gpsimd.dma_start(out=xout[tt * P:(tt + 1) * P, :], in_=yo)


IN_SPECS = {
    "x0": (L, C), "x1": (L, C),
    "Wqk": (C, C), "bqk": (C,), "Wv": (C, C), "bv": (C,),
    "Wout": (C, C), "bout": (C,),
    "W1": (C2, C2), "b1": (C2,), "ln_g": (C2,), "ln_b": (C2,),
    "W2": (C2, C), "b2": (C,),
}
OUT_SPECS = {"x0_out": (L, C), "x1_out": (L, C)}


def build_module():
    nc = bacc.Bacc("TRN2", target_bir_lowering=False)
    ins = {n: nc.dram_tensor(n, list(s), F32, kind="ExternalInput").ap()
           for n, s in IN_SPECS.items()}
    outs = {n: nc.dram_tensor(n, list(s), F32, kind="ExternalOutput").ap()
            for n, s in OUT_SPECS.items()}
    with tile.TileContext(nc) as tc, ExitStack() as ctx:
        cross_block(ctx, tc, ins, outs)
    nc.compile()
    return nc


_NC = None


def kernel(**inputs):
    global _NC
    if _NC is None:
        _NC = build_module()
    inp = {k: np.ascontiguousarray(np.asarray(v, dtype=np.float32))
           for k, v in inputs.items()}
    in_maps = []
    for b in range(B):
        m = {}
        for name in IN_SPECS:
            m[name] = inp[name][b] if name in ("x0", "x1") else inp[name]
        in_maps.append(m)
    res = run_bass_kernel_spmd(_NC, in_maps, list(range(B))).results
    x0o = np.stack([res[b]["x0_out"] for b in range(B)])
    x1o = np.stack([res[b]["x1_out"] for b in range(B)])
    return (x0o, x1o)

